# revision 1
# baseline (speedup 1.0000x reference)
"""DecoderRNNTAtt fused Trainium2 kernel - self-contained.

kernel(**inputs) takes the FULL unsharded inputs (as in reference.setup_inputs)
and returns the full [B, T, U, ODIM] float32 output, running on 8 NeuronCores.

Strategy: tensor-parallel recurrence (128-unit gate slices/core, one fused
AllGather per decoder step carrying q-partials + h0/h1 slices, layer-1 lagged
one step), attention context folded into the gate matmul via a block-diagonal
softmax-weight stationary, and batch-parallel joint network (core c owns batch
c) with the [T*U,JOINT]@[JOINT,ODIM] matmul interleaved one-u-per-step into
the collective windows. bf16 matmul operands, fp32 accumulation.
"""

import numpy as np
import ml_dtypes

import concourse.bass as bass
import concourse.mybir as mybir
import concourse.tile as tile
from concourse.tile import add_dep_helper
from concourse import bacc

BF = ml_dtypes.bfloat16
F32 = mybir.dt.float32
BF16 = mybir.dt.bfloat16
I32 = mybir.dt.int32

B, T, U = 8, 128, 32
EPROJS, DUNITS, EMB, ATT, JOINT, ODIM = 1024, 1024, 512, 512, 1024, 4096
NCORE = 8
KE = EPROJS // 128
KD = DUNITS // 128
KJ = JOINT // 128
KA = ATT // 128
GS = 512              # per-core gate slice (4 gates x 128 units)
UBLK = 4
AF = mybir.ActivationFunctionType
ALU = mybir.AluOpType

SHARD = ATT + 128 + 128     # AG shard rows: qp[0:512], h0s[512:640], h1s[640:768]
NBLK = U // UBLK
JOINT_PRIO_OFFSET = 200
DEBUG_CONST_ZT = False
FENCE_JOINT_BEFORE_REC = True
FENCE_STAGE_BEFORE_JOINT = True
REPEAT = 1
JCHUNK0 = 8


def host_prep(inputs):
    f32 = np.float32
    hs = np.asarray(inputs["hs_pad"], f32)
    ys = np.asarray(inputs["ys_in_pad"], np.int32)
    W_ih0 = np.asarray(inputs["W_ih0"], f32)

    # hsT[k, p, b*T+t] = hs[b, t, 128k+p]
    hsT = np.ascontiguousarray(hs.transpose(2, 0, 1).reshape(KE, 128, B * T)).astype(BF)

    shared = {
        "hsT": hsT,
        "embed_bf": np.asarray(inputs["embed"], f32).astype(BF),
        "ys_perm": np.ascontiguousarray(ys.T).reshape(-1),
        "hlens": np.asarray(inputs["hlens"], np.int32),
        "w_enc_bf": np.asarray(inputs["w_enc"], f32).astype(BF),
        "w_je_bf": np.asarray(inputs["w_je"], f32).astype(BF),
        "w_jd_bf": np.asarray(inputs["w_jd"], f32).astype(BF),
        "w_jo_bf": np.asarray(inputs["w_jo"], f32).astype(BF),
        "v_att_r": np.ascontiguousarray(
            np.asarray(inputs["v_att"], f32).reshape(KA, 128).T).astype(BF),
        "b_att_r": np.ascontiguousarray(
            np.asarray(inputs["b_att"], f32).reshape(KA, 128).T),
        "b_je_r": np.ascontiguousarray(
            np.asarray(inputs["b_je"], f32).reshape(KJ, 128).T),
        "b_jo": np.asarray(inputs["b_jo"], f32).astype(BF),
        "id32": np.eye(128, dtype=f32),
        "idbf": np.eye(128, dtype=f32).astype(BF),
    }

    per_core = []
    for c in range(NCORE):
        idx = np.concatenate([g * DUNITS + c * 128 + np.arange(128)
                              for g in range(4)])
        selb = np.zeros((128, KD * B), f32)
        selb[:, c::B] = 1.0
        pc = {
            "hsTb": np.ascontiguousarray(hsT[:, :, c * T:(c + 1) * T]),
            "selb": selb,
            "w_dec_sl": np.ascontiguousarray(
                np.asarray(inputs["w_dec"], f32)[c * 128:(c + 1) * 128, :]).astype(BF),
            "WeT": np.ascontiguousarray(W_ih0[idx, :EMB].T).astype(BF),
            "WaT": np.ascontiguousarray(W_ih0[idx, EMB:].T).astype(BF),
            "Whh0T": np.ascontiguousarray(np.asarray(inputs["W_hh0"], f32)[idx].T).astype(BF),
            "Wih1T": np.ascontiguousarray(np.asarray(inputs["W_ih1"], f32)[idx].T).astype(BF),
            "Whh1T": np.ascontiguousarray(np.asarray(inputs["W_hh1"], f32)[idx].T).astype(BF),
            "b0_s": np.ascontiguousarray(np.asarray(inputs["b0"], f32)[idx]),
            "b1_s": np.ascontiguousarray(np.asarray(inputs["b1"], f32)[idx]),
        }
        per_core.append({**shared, **pc})
    return per_core


def build_nc():
    nc = bacc.Bacc("TRN2", target_bir_lowering=False, debug=False,
                   num_devices=NCORE)

    def din(name, shape, dt):
        return nc.dram_tensor(name, shape, dt, kind="ExternalInput").ap()

    D = dict(
        hsT=din("hsT", [KE, 128, B * T], BF16),
        hsTb=din("hsTb", [KE, 128, T], BF16),
        selb=din("selb", [128, KD * B], F32),
        embed=din("embed_bf", [ODIM, EMB], BF16),
        ysp=din("ys_perm", [B * U], I32),
        hlens=din("hlens", [B], I32),
        wenc=din("w_enc_bf", [EPROJS, ATT], BF16),
        wje=din("w_je_bf", [EPROJS, JOINT], BF16),
        wjd=din("w_jd_bf", [DUNITS, JOINT], BF16),
        wjo=din("w_jo_bf", [JOINT, ODIM], BF16),
        vatt=din("v_att_r", [128, KA], BF16),
        batt=din("b_att_r", [128, KA], F32),
        bje=din("b_je_r", [128, KJ], F32),
        bjo=din("b_jo", [ODIM], BF16),
        id32=din("id32", [128, 128], F32),
        idbf=din("idbf", [128, 128], BF16),
        wdec=din("w_dec_sl", [128, ATT], BF16),
        WeT=din("WeT", [EMB, GS], BF16),
        WaT=din("WaT", [EPROJS, GS], BF16),
        Whh0T=din("Whh0T", [DUNITS, GS], BF16),
        Wih1T=din("Wih1T", [DUNITS, GS], BF16),
        Whh1T=din("Whh1T", [DUNITS, GS], BF16),
        b0=din("b0_s", [GS], F32),
        b1=din("b1_s", [GS], F32),
        z=nc.dram_tensor("z", [128, U * ODIM], F32, kind="ExternalOutput").ap(),
    )

    with tile.TileContext(nc) as tc:
        _emit(nc, tc, D)
    nc.compile()
    return nc


def _emit(nc, tc, D):
    with (
        tc.tile_pool(name="res", bufs=1) as res,
        tc.tile_pool(name="ps_g", bufs=3, space="PSUM") as ps_g,
        tc.tile_pool(name="ps_misc", bufs=2, space="PSUM") as ps_misc,
        tc.tile_pool(name="ps_big", bufs=3, space="PSUM") as ps_big,
        tc.tile_pool(name="dram", bufs=1, space="DRAM") as dram,
    ):
        _emit_inner(nc, tc, D, res, ps_g, ps_misc, ps_big, dram)


def _emit_inner(nc, tc, D, res, ps_g, ps_misc, ps_big, dram):
    # ---------------- resident tiles ----------------
    preT = [res.tile([128, B * 128], BF16, tag=f"preT{a}", name=f"preT{a}") for a in range(KA)]
    P_sb = [res.tile([128, GS], BF16, tag=f"P{b}", name=f"P{b}") for b in range(B)]
    jeT = [res.tile([128, 128], BF16, tag=f"jeT{j}", name=f"jeT{j}") for j in range(KJ)]
    G_ey = [res.tile([128, GS], BF16, tag=f"Gey{m}", name=f"Gey{m}") for m in range(2)]
    Whh0T = res.tile([128, KD * GS], BF16, tag="Whh0T", name="Whh0T")
    Wih1T = res.tile([128, KD * GS], BF16, tag="Wih1T", name="Wih1T")
    Whh1T = res.tile([128, KD * GS], BF16, tag="Whh1T", name="Whh1T")
    wdec = res.tile([128, ATT], BF16, tag="wdec", name="wdec")
    wjd = res.tile([128, KD * JOINT], BF16, tag="wjd", name="wjd")
    wjo = res.tile([128, KJ * ODIM], BF16, tag="wjo", name="wjo")
    vatt = res.tile([128, KA], BF16, tag="vatt", name="vatt")
    batt = res.tile([128, KA], F32, tag="batt", name="batt")
    bje = res.tile([128, KJ], F32, tag="bje", name="bje")
    bjob = res.tile([128, ODIM], BF16, tag="bjob", name="bjob")
    id32 = res.tile([128, 128], F32, tag="id32", name="id32")
    idbf = res.tile([128, 128], BF16, tag="idbf", name="idbf")
    vbd = res.tile([128, KA * B * B], BF16, tag="vbd", name="vbd")
    b1b = res.tile([B, GS], F32, tag="b1b", name="b1b")
    maskb = res.tile([B, T], F32, tag="maskb", name="maskb")
    selb = res.tile([128, KD * B], F32, tag="selb", name="selb")
    bd = res.tile([128, B * B], BF16, tag="bd", name="bd")
    c0 = res.tile([B, 128], F32, tag="c0", name="c0")
    c1 = res.tile([B, 128], F32, tag="c1", name="c1")
    hdecT = res.tile([128, KD * U], BF16, tag="hdecT", name="hdecT")
    zst = res.tile([128, 48], F32, tag="zst", name="zst")


    nc.sync.dma_start(Whh0T[:].rearrange("p (k n) -> p k n", n=GS), D["Whh0T"].rearrange("(k p) n -> p k n", p=128))
    nc.sync.dma_start(Wih1T[:].rearrange("p (k n) -> p k n", n=GS), D["Wih1T"].rearrange("(k p) n -> p k n", p=128))
    nc.sync.dma_start(Whh1T[:].rearrange("p (k n) -> p k n", n=GS), D["Whh1T"].rearrange("(k p) n -> p k n", p=128))
    nc.sync.dma_start(wdec[:], D["wdec"])
    nc.sync.dma_start(vatt[:], D["vatt"])
    nc.sync.dma_start(batt[:], D["batt"])
    nc.sync.dma_start(bje[:], D["bje"])
    nc.sync.dma_start(bjob[:], D["bjo"].unsqueeze(0).broadcast_to([128, ODIM]))
    nc.sync.dma_start(id32[:], D["id32"])
    nc.sync.dma_start(idbf[:], D["idbf"])
    nc.sync.dma_start(b1b[:], D["b1"].unsqueeze(0).broadcast_to([B, GS]))
    nc.sync.dma_start(selb[:], D["selb"])

    nc.sync.dma_start(wjd[:].rearrange("p (k n) -> p k n", n=JOINT), D["wjd"].rearrange("(k p) n -> p k n", p=128))
    nc.sync.dma_start(wjo[:].rearrange("p (k n) -> p k n", n=ODIM), D["wjo"].rearrange("(k p) n -> p k n", p=128))
    nc.gpsimd.memset(bd[:], 0.0)
    nc.gpsimd.memset(c0[:], 0.0)
    nc.gpsimd.memset(c1[:], 0.0)
    nc.gpsimd.memset(hdecT[:], 0.0)
    nc.gpsimd.memset(zst[:], 0.0)

    # ---------------- setup ----------------
    with tc.tile_pool(name="setup", bufs=1) as st:

        hsT = [st.tile([128, B * T], BF16, tag=f"hsT{k}", name=f"hsT{k}") for k in range(KE)]
        for k in range(KE):
            nc.sync.dma_start(hsT[k][:], D["hsT"][k])
        hsTb = [st.tile([128, T], BF16, tag=f"hsTb{k}", name=f"hsTb{k}") for k in range(KE)]
        for k in range(KE):
            nc.sync.dma_start(hsTb[k][:], D["hsTb"][k])
        wje = st.tile([128, KE * JOINT], BF16, tag="wje", name="wje")
        nc.sync.dma_start(wje[:].rearrange("p (k n) -> p k n", n=JOINT), D["wje"].rearrange("(k p) n -> p k n", p=128))
        wenc = st.tile([128, KE * ATT], BF16, tag="wenc", name="wenc")
        nc.sync.dma_start(wenc[:].rearrange("p (k n) -> p k n", n=ATT), D["wenc"].rearrange("(k p) n -> p k n", p=128))
        WaT = st.tile([128, KE * GS], BF16, tag="WaT", name="WaT")
        nc.sync.dma_start(WaT[:].rearrange("p (k n) -> p k n", n=GS), D["WaT"].rearrange("(k p) n -> p k n", p=128))
        WeT = st.tile([128, 4 * GS], BF16, tag="WeT", name="WeT")
        nc.sync.dma_start(WeT[:].rearrange("p (k n) -> p k n", n=GS), D["WeT"].rearrange("(k p) n -> p k n", p=128))
        b0b = st.tile([128, GS], F32, tag="b0b", name="b0b")
        nc.sync.dma_start(b0b[:], D["b0"].unsqueeze(0).broadcast_to([128, GS]))

        nc.gpsimd.memset(vbd[:], 0.0)
        for a in range(KA):
            for b in range(B):
                nc.vector.tensor_copy(vbd[:, (a * B + b) * B + b:(a * B + b) * B + b + 1],
                                      vatt[:, a:a + 1])

        # mask: maskb[b, t] = (t >= max(hlens[b],1)) * -1e9
        hl_i = st.tile([B, 1], I32, tag="hl_i", name="hl_i")
        nc.sync.dma_start(hl_i[:], D["hlens"].unsqueeze(1))
        hl_f = st.tile([B, 1], F32, tag="hl_f", name="hl_f")
        nc.vector.tensor_copy(hl_f[:], hl_i[:])
        nc.vector.tensor_scalar_max(hl_f[:], hl_f[:], 1.0)
        io_i = st.tile([B, T], I32, tag="io_i", name="io_i")
        nc.gpsimd.iota(io_i[:], pattern=[[1, T]], base=0, channel_multiplier=0)
        io_f = st.tile([B, T], F32, tag="io_f", name="io_f")
        nc.vector.tensor_copy(io_f[:], io_i[:])
        nc.vector.tensor_scalar(maskb[:], io_f[:], hl_f[:], None, op0=ALU.is_ge)
        nc.vector.tensor_scalar_mul(maskb[:], maskb[:], -1.0e9)

        # embedding gather (row r = u*8+b) + transpose
        ysp = st.tile([128, 2], I32, tag="ysp", name="ysp")
        nc.sync.dma_start(ysp[:], D["ysp"].rearrange("(m p) -> p m", p=128))
        eyT = [[None] * 4 for _ in range(2)]
        for m in range(2):
            ey = st.tile([128, EMB], BF16, tag=f"ey{m}", name=f"ey{m}")
            nc.gpsimd.indirect_dma_start(
                out=ey[:], out_offset=None, in_=D["embed"],
                in_offset=bass.IndirectOffsetOnAxis(ap=ysp[:, m:m + 1], axis=0))
            for e in range(4):
                tp = ps_misc.tile([128, 128], BF16, tag="ps_small", name="ps_small")
                nc.tensor.transpose(tp[:], ey[:, e * 128:(e + 1) * 128], idbf[:])
                eyT[m][e] = st.tile([128, 128], BF16, tag=f"eyT{m}_{e}", name=f"eyT{m}_{e}")
                nc.vector.tensor_copy(eyT[m][e][:], tp[:])

        # G_ey[m] = eyT[m].T @ WeT + b0
        for m in range(2):
            gp = ps_g.tile([128, GS], F32, tag="psg", name="psg")
            for e in range(4):
                nc.tensor.matmul(gp[:], eyT[m][e][:], WeT[:, e * GS:(e + 1) * GS],
                                 start=(e == 0), stop=(e == 3))
            nc.vector.tensor_tensor(out=G_ey[m][:], in0=gp[:], in1=b0b[:], op=ALU.add)

        # preT[a][:, b*128+t] = w_enc.T @ hsT + b_att
        for a in range(KA):
            for h in range(2):
                pp = ps_big.tile([128, 512], F32, tag="psb", name="psb")
                for k in range(KE):
                    nc.tensor.matmul(
                        pp[:], wenc[:, k * ATT + a * 128: k * ATT + (a + 1) * 128],
                        hsT[k][:, h * 512:(h + 1) * 512],
                        start=(k == 0), stop=(k == KE - 1))
                nc.scalar.activation(preT[a][:, h * 512:(h + 1) * 512], pp[:],
                                     AF.Identity, bias=batt[:, a:a + 1])

        # P_sb[b] = hs[b] @ Wa_s.T  -> [T, GS]
        for b in range(B):
            pp = ps_big.tile([128, GS], F32, tag="psb", name="psb")
            for k in range(KE):
                nc.tensor.matmul(pp[:], hsT[k][:, b * 128:(b + 1) * 128],
                                 WaT[:, k * GS:(k + 1) * GS],
                                 start=(k == 0), stop=(k == KE - 1))
            nc.vector.tensor_copy(P_sb[b][:], pp[:])

        # jeT[j] = (w_je.T @ hs[own].T)[128j..] + b_je
        for j in range(KJ):
            pp = ps_misc.tile([128, 128], F32, tag="ps_small", name="ps_small")
            for k in range(KE):
                nc.tensor.matmul(
                    pp[:], wje[:, k * JOINT + j * 128: k * JOINT + (j + 1) * 128],
                    hsTb[k][:], start=(k == 0), stop=(k == KE - 1))
            nc.scalar.activation(jeT[j][:], pp[:], AF.Identity, bias=bje[:, j:j + 1])

    # ---------------- main loop ----------------
    with (
        tc.tile_pool(name="mp", bufs=2) as mp,
        tc.tile_pool(name="ztp", bufs=2) as ztp,
        tc.tile_pool(name="ost", bufs=4) as ostp,
    ):
        env = dict(mp=mp, ztp=ztp, ostp=ostp, ps_g=ps_g, ps_misc=ps_misc,
                   ps_big=ps_big, dram=dram, res=res, preT=preT, P_sb=P_sb,
                   jeT=jeT, G_ey=G_ey, Whh0T=Whh0T, Wih1T=Wih1T, Whh1T=Whh1T,
                   wdec=wdec, wjd=wjd, wjo=wjo, vatt=vatt, batt=batt, bje=bje,
                   bjob=bjob, id32=id32, idbf=idbf, vbd=vbd, b1b=b1b,
                   maskb=maskb, selb=selb, bd=bd, c0=c0, c1=c1, hdecT=hdecT,
                   zst=zst)
        for _rep in range(REPEAT):
            _main_rep(nc, tc, D, env)


def _main_rep(nc, tc, D, env):
    (mp, ztp, ostp, ps_g, ps_misc, ps_big, dram, res) = (
        env["mp"], env["ztp"], env["ostp"], env["ps_g"], env["ps_misc"],
        env["ps_big"], env["dram"], env["res"])
    (preT, P_sb, jeT, G_ey, Whh0T, Wih1T, Whh1T, wdec, wjd, wjo, vatt, batt,
     bje, bjob, id32, idbf, vbd, b1b, maskb, selb, bd, c0, c1, hdecT, zst) = (
        env["preT"], env["P_sb"], env["jeT"], env["G_ey"], env["Whh0T"],
        env["Wih1T"], env["Whh1T"], env["wdec"], env["wjd"], env["wjo"],
        env["vatt"], env["batt"], env["bje"], env["bjob"], env["id32"],
        env["idbf"], env["vbd"], env["b1b"], env["maskb"], env["selb"],
        env["bd"], env["c0"], env["c1"], env["hdecT"], env["zst"])
    zts = [None, None]
    prev_js = None
    for k in range(U + 1):
        first = {}
        # ---- unpack AG(k-1) ----
        if k > 0:
            qps = mp.tile([128, NCORE * KA * B], F32, tag="qps", name="qps")
            first["sp"] = nc.sync.dma_start(
                qps[:].rearrange("p (g x) -> p g x", x=32),
                ag_out[:].rearrange("(g p) x -> p g x", p=128)[:, :, 16:48])
            qT = mp.tile([128, KA * B], F32, tag="qT", name="qT")
            first["dve"] = nc.vector.tensor_tensor(out=qT[:], in0=qps[:, 0:32], in1=qps[:, 32:64], op=ALU.add)
            for cc in range(2, NCORE):
                nc.vector.tensor_tensor(out=qT[:], in0=qT[:],
                                        in1=qps[:, cc * 32:(cc + 1) * 32], op=ALU.add)
            h0f = mp.tile([128, KD * B], F32, tag="h0f", name="h0f")
            nc.sync.dma_start(h0f[:].rearrange("p (g j) -> p g j", j=B),
                              _ag_h_ap(ag_out, 0))
            h1f = mp.tile([128, KD * B], F32, tag="h1f", name="h1f")
            nc.sync.dma_start(h1f[:].rearrange("p (g j) -> p g j", j=B),
                              _ag_h_ap(ag_out, 8))
            h0b = mp.tile([128, KD * B], BF16, tag="h0b", name="h0b")
            nc.vector.tensor_copy(h0b[:], h0f[:])
            h1b = mp.tile([128, KD * B], BF16, tag="h1b", name="h1b")
            nc.vector.tensor_copy(h1b[:], h1f[:])
            if k >= 2:
                hsel = mp.tile([128, KD * B], F32, tag="hsel", name="hsel")
                nc.vector.tensor_tensor(out=hsel[:], in0=h1f[:], in1=selb[:], op=ALU.mult)
                hred = mp.tile([128, KD], F32, tag="hred", name="hred")
                nc.vector.tensor_reduce(
                    out=hred[:].rearrange("p (g o) -> p g o", o=1),
                    in_=hsel[:].rearrange("p (g j) -> p g j", j=B),
                    op=ALU.add, axis=mybir.AxisListType.X)
                nc.vector.tensor_copy(
                    hdecT[:].rearrange("p (g u) -> p g u", u=U)[:, :, k - 2:k - 1],
                    hred[:].rearrange("p (g o) -> p g o", o=1))
        else:
            qT = mp.tile([128, KA * B], F32, tag="qT", name="qT")
            nc.gpsimd.memset(qT[:], 0.0)
            h0b = mp.tile([128, KD * B], BF16, tag="h0b", name="h0b")
            nc.gpsimd.memset(h0b[:], 0.0)
            h1b = mp.tile([128, KD * B], BF16, tag="h1b", name="h1b")
            nc.gpsimd.memset(h1b[:], 0.0)

        if False and FENCE_JOINT_BEFORE_REC:
            for eng in ("sp", "dve", "pe", "act"):
                if eng in first and eng in prev_joint:
                    add_dep_helper(first[eng].ins, prev_joint[eng].ins,
                                   sync=False,
                                   reason="joint fills AG window")
        S = mp.tile([128, 48], F32, tag="S", name="S")

        # ---- L1 for step k-1 ----
        if k >= 1:
            g1 = ps_g.tile([B, GS], F32, tag="psg", name="psg")
            for kk in range(KD):
                mm = nc.tensor.matmul(g1[:], h0b[:, kk * B:(kk + 1) * B],
                                      Wih1T[:, kk * GS:(kk + 1) * GS],
                                      start=(kk == 0), stop=False)
                if kk == 0:
                    first["pe"] = mm
            for kk in range(KD):
                nc.tensor.matmul(g1[:], h1b[:, kk * B:(kk + 1) * B],
                                 Whh1T[:, kk * GS:(kk + 1) * GS],
                                 start=False, stop=(kk == KD - 1))
            _lstm_tail(nc, mp, ps_misc, g1, b1b[:], c1, id32, S, 8, "g1")

        # ---- attention + L0 for step k ----
        if k <= U - 1:
            tanh_sb = [ztp.tile([128, B * 128], BF16, tag=f"tanh{a}",
                                name=f"tanh{a}") for a in range(KA)]
            for a in range(KA):
                for b in range(B):
                    aa = nc.scalar.activation(
                        tanh_sb[a][:, b * 128:(b + 1) * 128],
                        preT[a][:, b * 128:(b + 1) * 128],
                        AF.Tanh, bias=qT[:, a * B + b: a * B + b + 1])
                    if a == 0 and b == 0:
                        first["act"] = aa
            e_ps = ps_misc.tile([B, T], F32, tag="ps_small", name="e_ps")
            for a in range(KA):
                for b in range(B):
                    nc.tensor.matmul(
                        e_ps[:],
                        vbd[:, (a * B + b) * B:(a * B + b + 1) * B],
                        tanh_sb[a][:, b * 128:(b + 1) * 128],
                        start=(b == 0 and a == 0),
                        stop=(b == B - 1 and a == KA - 1))
            _joint_chunks(nc, prev_js, 1, ostp, ps_big, wjo, bjob, D["z"])
            e8 = mp.tile([B, T], F32, tag="e8", name="e8")
            nc.vector.tensor_tensor(out=e8[:], in0=e_ps[:], in1=maskb[:], op=ALU.add)
            mx = mp.tile([B, 1], F32, tag="mx", name="mx")
            nc.vector.reduce_max(out=mx[:], in_=e8[:],
                                 axis=mybir.AxisListType.X, negate=True)
            ssum = mp.tile([B, 1], F32, tag="ssum", name="ssum")
            w8 = mp.tile([B, T], F32, tag="w8", name="w8")
            nc.scalar.activation(w8[:], e8[:], AF.Exp, bias=mx[:], accum_out=ssum[:])
            rs = mp.tile([B, 1], F32, tag="rs", name="rs")
            nc.vector.reciprocal(rs[:], ssum[:])
            nc.vector.tensor_scalar_mul(w8[:], w8[:], rs[:])
            wt_ps = ps_misc.tile([128, B], F32, tag="ps_small", name="ps_small")
            nc.tensor.transpose(wt_ps[:], w8[:], id32[0:B, 0:B])
            for b in range(B):
                nc.vector.tensor_copy(bd[:, b * B + b:b * B + b + 1],
                                      wt_ps[:, b:b + 1])

            g0 = ps_g.tile([B, GS], F32, tag="psg", name="psg")
            for kk in range(KD):
                nc.tensor.matmul(g0[:], h0b[:, kk * B:(kk + 1) * B],
                                 Whh0T[:, kk * GS:(kk + 1) * GS],
                                 start=(kk == 0), stop=False)
            mm_gey = nc.tensor.matmul(
                g0[:], idbf[:, (k % 16) * B:(k % 16) * B + B],
                G_ey[k // 16][:], start=False, stop=False)
            _joint_chunks(nc, prev_js, 5, ostp, ps_big, wjo, bjob,
                          D["z"], fence=mm_gey)
            for b in range(B):
                nc.tensor.matmul(g0[:], bd[:, b * B:(b + 1) * B], P_sb[b][:],
                                 start=False, stop=(b == B - 1))
            _lstm_tail(nc, mp, ps_misc, g0, None, c0, id32, S, 0, "g0")

            h0sT_bf = mp.tile([128, B], BF16, tag="h0sTb", name="h0sTb")
            nc.vector.tensor_copy(h0sT_bf[:], S[:, 0:8])
            for a in range(KA):
                qp = ps_misc.tile([128, B], F32, tag="ps_small", name="ps_small")
                nc.tensor.matmul(qp[:], wdec[:, a * 128:(a + 1) * 128],
                                 h0sT_bf[:], start=True, stop=True)
                nc.vector.tensor_copy(S[:, 16 + a * B:16 + (a + 1) * B], qp[:])

        # ---- joint prep (jd/jdT) for the next u-block ----
        if k >= 5 and (k - 5) % UBLK == 0 and (k - 5) // UBLK < NBLK - 1:
            j = (k - 5) // UBLK
            zts[j % 2] = _joint_prep(nc, j, mp, ztp, ps_g, ps_misc,
                                     hdecT, wjd, jeT, id32)

        # ---- stage + AllGather (shard [128, 48]: h0 0:8, h1 8:16, qpT 16:48) ----
        ag_in = dram.tile([128, 48], F32, tag="ag_in", name="ag_in")
        ag_out_new = dram.tile([NCORE * 128, 48], F32, tag="ag_out",
                               name="ag_out", addr_space="Shared")
        if k == 0:
            nc.vector.tensor_copy(S[:, 8:16], zst[:, 8:16])
        if k == U:
            nc.vector.tensor_copy(S[:, 0:8], zst[:, 0:8])
            nc.vector.tensor_copy(S[:, 16:48], zst[:, 16:48])
        i_stage = nc.sync.dma_start(ag_in[:], S[:])
        nc.gpsimd.collective_compute(
            "AllGather", ALU.bypass, ins=[ag_in[:].opt()],
            outs=[ag_out_new[:].opt()], replica_groups=[list(range(NCORE))])
        ag_out = ag_out_new

        # ---- joint: tanh for u=k-5, 2 odim-chunks into the AG window,
        # remainder spread across the next superstep's PE bubbles ----
        if 5 <= k <= 4 + U - UBLK:
            prev_js = _joint_u_tanh(nc, k - 5, zts[((k - 5) // UBLK) % 2],
                                    ztp, jeT)
            _joint_chunks(nc, prev_js, JCHUNK0, ostp, ps_big, wjo, bjob,
                          D["z"], fence=i_stage)



    # epilogue: unpack last AG's h1, final joint block
    h1f = mp.tile([128, KD * B], F32, tag="h1f", name="h1f")
    nc.sync.dma_start(h1f[:].rearrange("p (g j) -> p g j", j=B),
                      _ag_h_ap(ag_out, 8))
    hsel = mp.tile([128, KD * B], F32, tag="hsel", name="hsel")
    nc.vector.tensor_tensor(out=hsel[:], in0=h1f[:], in1=selb[:], op=ALU.mult)
    hred = mp.tile([128, KD], F32, tag="hred", name="hred")
    nc.vector.tensor_reduce(
        out=hred[:].rearrange("p (g o) -> p g o", o=1),
        in_=hsel[:].rearrange("p (g j) -> p g j", j=B),
        op=ALU.add, axis=mybir.AxisListType.X)
    nc.vector.tensor_copy(
        hdecT[:].rearrange("p (g u) -> p g u", u=U)[:, :, U - 1:U],
        hred[:].rearrange("p (g o) -> p g o", o=1))
    _joint_chunks(nc, prev_js, 8, ostp, ps_big, wjo, bjob, D["z"])
    zts[(NBLK - 1) % 2] = _joint_prep(nc, NBLK - 1, mp, ztp, ps_g, ps_misc,
                                      hdecT, wjd, jeT, id32)
    for u in range(U - UBLK, U):
        js = _joint_u_tanh(nc, u, zts[(u // UBLK) % 2], ztp, jeT)
        _joint_chunks(nc, js, 8, ostp, ps_big, wjo, bjob, D["z"])


def _ag_h_ap(ag_out, off):
    # [p, g, j] <- ag_out[g*128 + p, off + j]  (shard g holds unit-slice g)
    return ag_out[:].rearrange("(g p) x -> p g x", p=128)[:, :, off:off + 8]


def _lstm_tail(nc, mp, ps_misc, g_ps, bias_rows, c_state, id32, S, scol, tag):
    if bias_rows is not None:
        gsb = mp.tile([B, GS], F32, tag=f"{tag}_sb", name=f"{tag}_sb")
        nc.vector.tensor_tensor(out=gsb[:], in0=g_ps[:], in1=bias_rows, op=ALU.add)
    else:
        gsb = g_ps
    ga = mp.tile([B, GS], F32, tag=f"{tag}_act", name=f"{tag}_act")
    nc.scalar.activation(ga[:, 0:256], gsb[:, 0:256], AF.Sigmoid)
    nc.scalar.activation(ga[:, 256:384], gsb[:, 256:384], AF.Tanh)
    nc.scalar.activation(ga[:, 384:512], gsb[:, 384:512], AF.Sigmoid)
    t1 = mp.tile([B, 128], F32, tag=f"{tag}_t1", name=f"{tag}_t1")
    nc.vector.tensor_tensor(out=t1[:], in0=ga[:, 128:256], in1=c_state[:], op=ALU.mult)
    t2 = mp.tile([B, 128], F32, tag=f"{tag}_t2", name=f"{tag}_t2")
    nc.vector.tensor_tensor(out=t2[:], in0=ga[:, 0:128], in1=ga[:, 256:384], op=ALU.mult)
    nc.vector.tensor_tensor(out=c_state[:], in0=t1[:], in1=t2[:], op=ALU.add)
    tc_ = mp.tile([B, 128], F32, tag=f"{tag}_tc", name=f"{tag}_tc")
    nc.scalar.activation(tc_[:], c_state[:], AF.Tanh)
    hs_ = mp.tile([B, 128], F32, tag=f"{tag}_h", name=f"{tag}_h")
    nc.vector.tensor_tensor(out=hs_[:], in0=ga[:, 384:512], in1=tc_[:], op=ALU.mult)
    ht_ps = ps_misc.tile([128, B], F32, tag="ps_small", name="ps_small")
    nc.tensor.transpose(ht_ps[:], hs_[:], id32[0:B, 0:B])
    nc.vector.tensor_copy(S[:, scol:scol + 8], ht_ps[:])
    return hs_


def _joint_prep(nc, j, mp, ztp, ps_g, ps_misc, hdecT, wjd, jeT, id32):
    """jd -> jdT -> zt tanh tiles for u-block j. Returns zt tile list."""
    u0 = j * UBLK
    jd_sb = mp.tile([UBLK, JOINT], F32, tag="jd_sb", name="jd_sb")
    for n2 in range(2):
        jp = ps_g.tile([B, GS], F32, tag="psg", name="psg")
        for kk in range(KD):
            nc.tensor.matmul(
                jp[0:UBLK, :],
                hdecT[:, kk * U + u0: kk * U + u0 + UBLK],
                wjd[:, kk * JOINT + n2 * 512: kk * JOINT + (n2 + 1) * 512],
                start=(kk == 0), stop=(kk == KD - 1))
        nc.vector.tensor_copy(jd_sb[:, n2 * 512:(n2 + 1) * 512], jp[0:UBLK, :])
    jdt = mp.tile([128, KJ * UBLK], F32, tag="jdt", name="jdt", bufs=2)
    for j2 in range(KJ):
        tp = ps_misc.tile([128, B], F32, tag="ps_small", name="ps_small")
        nc.tensor.transpose(tp[0:128, 0:UBLK], jd_sb[:, j2 * 128:(j2 + 1) * 128],
                            id32[0:UBLK, 0:UBLK])
        nc.vector.tensor_copy(jdt[:, j2 * UBLK:(j2 + 1) * UBLK], tp[0:128, 0:UBLK])
    return jdt


def _joint_u_tanh(nc, u, jdt, ztp, jeT):
    uu = u % UBLK
    zt = [ztp.tile([128, 128], BF16, tag=f"ztu{j2}", name=f"ztu{j2}", bufs=3)
          for j2 in range(KJ)]
    for j2 in range(KJ):
        nc.scalar.activation(
            zt[j2][:], jeT[j2][:], AF.Tanh,
            bias=jdt[:, j2 * UBLK + uu: j2 * UBLK + uu + 1])
    return {"zt": zt, "u": u, "n": 0}


def _joint_chunks(nc, js, count, ostp, ps_big, wjo, bjob, z_d, fence=None):
    """Emit `count` odim-chunks of the big matmul for joint state js."""
    if js is None or js["n"] >= ODIM // 512:
        return None
    mm = None
    u, zt = js["u"], js["zt"]
    first = True
    for n in range(js["n"], min(js["n"] + count, ODIM // 512)):
        zb = ps_big.tile([128, 512], F32, tag="psb", name="psb")
        for kk in range(KJ):
            mm = nc.tensor.matmul(
                zb[:], zt[kk][:],
                wjo[:, kk * ODIM + n * 512: kk * ODIM + (n + 1) * 512],
                start=(kk == 0), stop=(kk == KJ - 1))
            if first and fence is not None:
                add_dep_helper(mm.ins, fence.ins, sync=False,
                               reason="joint chunk placement")
                first = False
        ost = ostp.tile([128, 512], F32, tag="ost", name="ost")
        nc.vector.tensor_tensor(out=ost[:], in0=zb[:],
                                in1=bjob[:, n * 512:(n + 1) * 512], op=ALU.add)
        nc.sync.dma_start(
            z_d[:, u * ODIM + n * 512: u * ODIM + (n + 1) * 512],
            ost[:])
    js["n"] = min(js["n"] + count, ODIM // 512)
    return mm


# ---- SPMD runner ----

import numpy as np
import jax
from jax.sharding import Mesh, PartitionSpec
from jax.experimental.shard_map import shard_map

import concourse.mybir as mybir
import concourse.bass as bass
from concourse import bass2jax
from concourse.bass2jax import _bass_exec_p, partition_id_tensor


def build_spmd_fn(nc: bass.Bass, n_cores: int):
    """Returns (fn, in_names, out_names). fn(in_maps) -> list of per-core out dicts."""
    bass2jax.install_neuronx_cc_hook()

    if nc.dbg_addr is not None and nc.dbg_callbacks:
        raise RuntimeError("debug callbacks unsupported")

    partition_name = nc.partition_id_tensor.name if nc.partition_id_tensor else None

    in_names, out_names, out_avals, zero_outs = [], [], [], []
    for alloc in nc.m.functions[0].allocations:
        if not isinstance(alloc, mybir.MemoryLocationSet):
            continue
        name = alloc.memorylocations[0].name
        if alloc.kind == "ExternalInput":
            if name != partition_name:
                in_names.append(name)
        elif alloc.kind == "ExternalOutput":
            out_names.append(name)
            shape = tuple(alloc.tensor_shape)
            dtype = mybir.dt.np(alloc.dtype)
            out_avals.append(jax.core.ShapedArray(shape, dtype))
            zero_outs.append(np.zeros(shape, dtype))
    n_params = len(in_names)
    n_outs = len(out_avals)
    all_in_names = list(in_names) + list(out_names)
    if partition_name is not None:
        all_in_names.append(partition_name)

    def _body(*args):
        operands = list(args)
        if partition_name is not None:
            operands.append(partition_id_tensor())
        outs = _bass_exec_p.bind(
            *operands,
            out_avals=tuple(out_avals),
            in_names=tuple(all_in_names),
            out_names=tuple(out_names),
            lowering_input_output_aliases=(),
            sim_require_finite=True,
            sim_require_nnan=True,
            nc=nc,
        )
        return tuple(outs)

    devices = jax.devices()[:n_cores]
    mesh = Mesh(np.asarray(devices), ("core",))
    in_specs = (PartitionSpec("core"),) * (n_params + n_outs)
    out_specs = (PartitionSpec("core"),) * n_outs
    donate = tuple(range(n_params, n_params + n_outs))
    sharded = jax.jit(
        shard_map(_body, mesh=mesh, in_specs=in_specs, out_specs=out_specs,
                  check_rep=False),
        donate_argnums=donate, keep_unused=True,
    )

    from jax.sharding import NamedSharding
    shard0 = NamedSharding(mesh, PartitionSpec("core"))

    def stage_inputs(in_maps):
        """Pre-transfer inputs to device; returns staged list usable in exec()."""
        per_core = [[np.asarray(m[name]) for name in in_names] for m in in_maps]
        concat_in = [np.concatenate([per_core[c][i] for c in range(n_cores)], axis=0)
                     for i in range(n_params)]
        staged = [jax.device_put(a, shard0) for a in concat_in]
        jax.block_until_ready(staged)
        return staged

    def stage_zeros():
        z = [jax.device_put(np.zeros((n_cores * s.shape[0], *s.shape[1:]), s.dtype), shard0)
             for s in zero_outs]
        jax.block_until_ready(z)
        return z

    def exec_staged(staged_in, staged_zeros, return_outputs=True):
        out_arrs = sharded(*staged_in, *staged_zeros)
        jax.block_until_ready(out_arrs)
        if not return_outputs:
            return None
        return [
            {name: np.asarray(out_arrs[i]).reshape(n_cores, *out_avals[i].shape)[c]
             for i, name in enumerate(out_names)}
            for c in range(n_cores)
        ]

    def fn(in_maps, return_outputs=True):
        return exec_staged(stage_inputs(in_maps), stage_zeros(), return_outputs)

    fn.stage_inputs = stage_inputs
    fn.stage_zeros = stage_zeros
    fn.exec_staged = exec_staged
    return fn, in_names, out_names


_CACHED = None


def _get_fn():
    global _CACHED
    if _CACHED is None:
        nc = build_nc()
        fn, _, _ = build_spmd_fn(nc, NCORE)
        _CACHED = fn
    return _CACHED


def kernel(**inputs):
    per_core = host_prep(inputs)
    fn = _get_fn()
    res = fn(per_core)
    out = np.stack([res[c]["z"].reshape(T, U, ODIM) for c in range(NCORE)])
    return out.astype(np.float32)

